# revision 1
# baseline (speedup 1.0000x reference)
import numpy as np
import concourse.bass as bass
import concourse.mybir as mybir
from concourse.bass_utils import run_bass_kernel_spmd

# hardcoded problem dims
B, N, BQ, BK = 2, 2048, 32, 128
NB = N // BQ
CS, CZ, CH, H, PQK, PV = 384, 128, 16, 12, 4, 8
INF, EPS = 1e5, 1e-8
NCORES = 8
BLK_PER_CORE = (B * NB) // NCORES  # 16


QG = 4                      # q-tiles per DMA group
NGRP = BQ // QG             # 8 groups per block
NBUF = 4


def _build_nc():
    """Per-core graph (raw bass, explicit semaphores): stream this core's z
    shard through SBUF computing per-row LayerNorm statistics (sum and
    sum-of-squares over the channel axis) on the vector engine, double
    buffered against the DMA stream."""
    nc = bass.Bass()
    zb = nc.dram_tensor("zb", [BLK_PER_CORE, BQ, BK, CZ], mybir.dt.float32,
                        kind="ExternalInput")
    out = nc.dram_tensor("out", [BLK_PER_CORE, BK, 2 * BQ], mybir.dt.float32,
                         kind="ExternalOutput")
    NB_ = BLK_PER_CORE

    with (
        nc.sbuf_tensor([BK, NBUF, QG * CZ], mybir.dt.float32) as zts,
        nc.sbuf_tensor([BK, QG * CZ], mybir.dt.float32) as sc,
        nc.sbuf_tensor([BK, 3, 2 * BQ], mybir.dt.float32) as stats,
        nc.semaphore() as dma_sem,
        nc.semaphore() as v_sem,
        nc.semaphore() as out_sem,
        nc.Block() as block,
    ):
        @block.sync
        def _(sync):
            it = 0
            for blk in range(NB_):
                for g in range(NGRP):
                    if it >= NBUF:
                        sync.wait_ge(v_sem, it - NBUF + 1)
                    src = zb[blk, g * QG:(g + 1) * QG, :, :].rearrange(
                        "a k c -> k a c")
                    dst = zts[:, it % NBUF, :].rearrange(
                        "k (a c) -> k a c", a=QG)
                    sync.dma_start(dst, src).then_inc(dma_sem, 16)
                    it += 1
                if blk >= 1:
                    b = blk - 1
                    sync.wait_ge(v_sem, NGRP * (b + 1))
                    sync.dma_start(
                        out[b, :, :], stats[:, b % 3, :]).then_inc(out_sem, 16)
            sync.wait_ge(v_sem, NGRP * NB_)
            sync.dma_start(
                out[NB_ - 1, :, :],
                stats[:, (NB_ - 1) % 3, :]).then_inc(out_sem, 16)

        @block.vector
        def _(vector):
            it = 0
            for blk in range(NB_):
                for g in range(NGRP):
                    vector.wait_ge(dma_sem, 16 * (it + 1))
                    if g == 0 and blk >= 3:
                        vector.wait_ge(out_sem, 16 * (blk - 2))
                    zview = zts[:, it % NBUF, :].rearrange(
                        "k (a c) -> k a c", a=QG)
                    nc.vector.tensor_reduce(
                        stats[:, blk % 3, g * QG:(g + 1) * QG], zview,
                        mybir.AxisListType.X, mybir.AluOpType.add)
                    nc.vector.scalar_tensor_tensor(
                        sc[:, :], zts[:, it % NBUF, :], 1.0,
                        zts[:, it % NBUF, :],
                        mybir.AluOpType.mult, mybir.AluOpType.mult)
                    nc.vector.tensor_reduce(
                        stats[:, blk % 3, BQ + g * QG:BQ + (g + 1) * QG],
                        sc[:, :].rearrange("k (a c) -> k a c", a=QG),
                        mybir.AxisListType.X,
                        mybir.AluOpType.add).then_inc(v_sem, 1)
                    it += 1
    return nc


def _softplus(x):
    return np.logaddexp(np.float32(0.0), x.astype(np.float32)).astype(np.float32)


def _run_device(z, trace=False):
    """z: [B*NB, BQ, BK, CZ] f32. Returns stats [B*NB, BK, 2*BQ], exec_ns."""
    nc = _build_nc()
    in_maps = []
    for i in range(NCORES):
        shard = np.ascontiguousarray(z[i * BLK_PER_CORE:(i + 1) * BLK_PER_CORE])
        in_maps.append({"zb": shard})
    try:
        res = run_bass_kernel_spmd(nc, in_maps, core_ids=list(range(NCORES)),
                                   trace=trace)
    except ModuleNotFoundError:
        res = run_bass_kernel_spmd(nc, in_maps, core_ids=list(range(NCORES)),
                                   trace=False)
    exec_ns = res.exec_time_ns
    if trace and exec_ns is None:
        # NTFF hook unavailable: wall-clock the cached executable as a bound
        import time
        t0 = time.perf_counter()
        res = run_bass_kernel_spmd(nc, in_maps, core_ids=list(range(NCORES)),
                                   trace=False)
        exec_ns = int((time.perf_counter() - t0) * 1e9)
    stats = np.concatenate([r["out"] for r in res.results], axis=0)
    return stats, exec_ns


def kernel(s, z, trans, rots, s_mask, key_idx,
           ln_s_g, ln_s_b, ln_z_g, ln_z_b,
           Wq, Wk, Wv, Wqp, Wkvp, Wb, Wdz, head_weights, Wout,
           _trace=False):
    f = np.float32
    s = np.asarray(s, f); z = np.asarray(z, f)
    trans = np.asarray(trans, f); rots = np.asarray(rots, f)
    s_mask = np.asarray(s_mask, f)
    key_idx = np.asarray(key_idx).astype(np.int64)
    ln_s_g = np.asarray(ln_s_g, f); ln_s_b = np.asarray(ln_s_b, f)
    ln_z_g = np.asarray(ln_z_g, f); ln_z_b = np.asarray(ln_z_b, f)
    Wq = np.asarray(Wq, f); Wk = np.asarray(Wk, f); Wv = np.asarray(Wv, f)
    Wqp = np.asarray(Wqp, f); Wkvp = np.asarray(Wkvp, f)
    Wb = np.asarray(Wb, f); Wdz = np.asarray(Wdz, f)
    head_weights = np.asarray(head_weights, f); Wout = np.asarray(Wout, f)

    # device: z row statistics (LayerNorm reductions) on 8 cores
    zblocks = z.reshape(B * NB, BQ, BK, CZ)
    stats, exec_ns = _run_device(zblocks, trace=_trace)
    if _trace:
        kernel._last_exec_ns = exec_ns
    sums = stats[:, :, :BQ].transpose(0, 2, 1).reshape(B, NB, BQ, BK)
    sumsq = stats[:, :, BQ:].transpose(0, 2, 1).reshape(B, NB, BQ, BK)
    m = sums / f(CZ)
    var = np.maximum(sumsq / f(CZ) - m * m, f(0.0))
    rr = f(1.0) / np.sqrt(var + f(1e-5))
    zN = (z - m[..., None]) * rr[..., None] * ln_z_g + ln_z_b

    # s-side LN
    mu = s.mean(-1, keepdims=True)
    v = ((s - mu) ** 2).mean(-1, keepdims=True)
    sN = (s - mu) / np.sqrt(v + f(1e-5)) * ln_s_g + ln_s_b

    q_in = sN.reshape(B, NB, BQ, CS)
    k_in = sN[:, key_idx]
    q_t = trans.reshape(B, NB, BQ, 3)
    q_R = rots.reshape(B, NB, BQ, 3, 3)
    k_t = trans[:, key_idx]
    k_R = rots[:, key_idx]

    q = (q_in @ Wq).reshape(B, NB, BQ, H, CH)
    k = (k_in @ Wk).reshape(B, NB, BK, H, CH)
    v_ = (k_in @ Wv).reshape(B, NB, BK, H, CH)

    q_pts = (q_in @ Wqp).reshape(B, NB, BQ, H * PQK, 3)
    q_pts = np.einsum('bnqij,bnqpj->bnqpi', q_R, q_pts) + q_t[..., None, :]
    q_pts = q_pts.reshape(B, NB, BQ, H, PQK, 3)
    kv_pts = (k_in @ Wkvp).reshape(B, NB, BK, H * (PQK + PV), 3)
    kv_pts = np.einsum('bnkij,bnkpj->bnkpi', k_R, kv_pts) + k_t[..., None, :]
    kv_pts = kv_pts.reshape(B, NB, BK, H, PQK + PV, 3)
    k_pts, v_pts = kv_pts[..., :PQK, :], kv_pts[..., PQK:, :]

    bbias = zN @ Wb
    a = np.einsum('bnqhc,bnkhc->bnqkh', q, k) * f(np.sqrt(1.0 / (3 * CH)))
    a = a + f(np.sqrt(1.0 / 3)) * bbias

    pt = f(-2.0) * np.einsum('bnqhpd,bnkhpd->bnqkh', q_pts, k_pts)
    qn = np.sum(q_pts ** 2, axis=(-1, -2))
    kn = np.sum(k_pts ** 2, axis=(-1, -2))
    pt = pt + qn[..., None, :] + kn[..., None, :, :]
    hw = _softplus(head_weights) * f(np.sqrt(1.0 / (3 * (PQK * 9.0 / 2))))
    pt = pt * hw * f(-0.5)
    a = a + pt

    q_mask = s_mask.reshape(B, NB, BQ)
    k_mask = s_mask[:, key_idx]
    am = q_mask[..., :, None] * k_mask[..., None, :]
    a = a + (INF * (am - f(1.0)))[..., None]
    a = np.swapaxes(a, -1, -2)
    a = a - a.max(-1, keepdims=True)
    a = np.exp(a)
    a = a / a.sum(-1, keepdims=True)

    o = np.einsum('bnqhk,bnkhc->bnqhc', a, v_).reshape(B, NB, BQ, H * CH)
    o_pt = np.einsum('bnqhk,bnkhvc->bnqhvc', a, v_pts)
    o_pt = np.einsum('bnqji,bnqhvj->bnqhvi', q_R,
                     o_pt - q_t[..., None, None, :])
    o_pt_d = np.sqrt(np.sum(o_pt ** 2, -1) + f(EPS)).reshape(B, NB, BQ, H * PV)
    o_pt_f = o_pt.reshape(B, NB, BQ, H * PV * 3)
    pair_z = zN @ Wdz
    o_pair = np.einsum('bnqhk,bnqkc->bnqhc', a, pair_z).reshape(
        B, NB, BQ, H * (CZ // 4))

    feats = np.concatenate([o, o_pt_f, o_pt_d, o_pair], -1)
    out = feats @ Wout
    return out.reshape(B, N, CS).astype(np.float32)



# revision 20
# speedup vs baseline: 5.7633x; 5.7633x over previous
"""BlockInvariantPointAttention on 8 trn2 cores — full on-device kernel.

Host does only: LayerNorm of s (6MB), small gathers/packs, weight folds.
Device (per core, 16 blocks): projections, per-row rotations, fused logit
matmuls, softmax, o/o_pt/o_pair contractions, final out-projection.

Key algebra (validated in model_check.py):
  zN @ W == rr * (z @ W~) + const,  W~ = (I - J/128) @ (diag(g) W);  the
  const and every per-(q,h)-row constant (qn, cb, masked-q) cancels in
  softmax or folds into bias columns.
"""
import numpy as np
import ml_dtypes

import concourse.bass as bass
import concourse.bacc as bacc
import concourse.mybir as mybir
import concourse.tile as tile
from concourse.bass_utils import run_bass_kernel_spmd
from concourse.masks import make_identity
from concourse.alu_op_type import AluOpType

f32 = np.float32
bf16 = ml_dtypes.bfloat16
FP = mybir.dt.float32
BF = mybir.dt.bfloat16
AX = mybir.AxisListType.X
AF = mybir.ActivationFunctionType
MUL = AluOpType.mult
ADD = AluOpType.add
SUB = AluOpType.subtract

# problem dims
B, N, BQ, BK = 2, 2048, 32, 128
NB = N // BQ
CS, CZ, CH, H, PQK, PV = 384, 128, 16, 12, 4, 8
INF, EPS = 1e5, 1e-8
NCORES = 8
BLK = (B * NB) // NCORES          # 16 blocks per core
NQW = 336                          # q-side proj width  (192 q + 144 qp)
NKV = 384                          # k-side [Wk|Wv]
NKP = 432                          # k-side Wkvp width
NF0 = 576                          # row-major feats (o, o_pt_f, o_pt_d)


def _bc(ap, dim, count):
    """Insert a step-0 (broadcast) dim at position `dim` of an AP."""
    new = [list(d) for d in ap.ap]
    new.insert(dim, [0, count])
    return bass.AP(ap.tensor, ap.offset, new)


def build_nc(nblk=BLK):
    nc = bacc.Bacc(None, target_bir_lowering=False)
    zb = nc.dram_tensor("zb", [nblk, BQ, BK, CZ], BF, kind="ExternalInput")
    qkT = nc.dram_tensor("qkT", [nblk, CS, BQ + BK], BF, kind="ExternalInput")
    wcat = nc.dram_tensor("wcat", [3, 128, NQW + NKV + NKP], BF,
                          kind="ExternalInput")
    wz = nc.dram_tensor("wz", [CZ, 44], BF, kind="ExternalInput")
    wout = nc.dram_tensor("wout", [8, 128, CS], BF, kind="ExternalInput")
    rotq = nc.dram_tensor("rotq", [nblk, BQ, 12], FP, kind="ExternalInput")
    rotk = nc.dram_tensor("rotk", [nblk, BK, 12], FP, kind="ExternalInput")
    hwq = nc.dram_tensor("hwq", [128, 4], FP, kind="ExternalInput")
    hwm = nc.dram_tensor("hwm", [128, H], FP, kind="ExternalInput")
    kmask = nc.dram_tensor("kmask", [nblk, H, BK], FP, kind="ExternalInput")
    bdz = nc.dram_tensor("bdz", [BQ, 1], FP, kind="ExternalInput")
    NQ = nblk * BQ
    outT = nc.dram_tensor("outT", [3, 128, NQ], FP, kind="ExternalOutput")

    with tile.TileContext(nc) as tc:
        import contextlib
        with contextlib.ExitStack() as ctx:
            ones = ctx.enter_context(tc.tile_pool(name="ones", bufs=1))
            wcat_sb = ones.tile([128, 3, NQW + NKV + NKP], BF)
            nc.sync.dma_start(wcat_sb, wcat.rearrange("a p n -> p a n"))
            wz_sb = ones.tile([CZ, 44], BF)
            nc.sync.dma_start(wz_sb, wz[:])
            wout_sb = ones.tile([128, 8, CS], BF)
            nc.sync.dma_start(wout_sb, wout.rearrange("a p n -> p a n"))
            hwq_sb = ones.tile([128, 4], FP)
            nc.sync.dma_start(hwq_sb, hwq[:])
            hwm_sb = ones.tile([128, H], FP)
            nc.sync.dma_start(hwm_sb, hwm[:])
            bdz_sb = ones.tile([BQ, 1], FP)
            nc.sync.dma_start(bdz_sb, bdz[:])
            rotq_sb = ones.tile([BQ, nblk, 12], FP)
            nc.sync.dma_start(rotq_sb, rotq.rearrange("b p n -> p b n"))
            rotk_sb = ones.tile([BK, nblk, 12], FP)
            nc.sync.dma_start(rotk_sb, rotk.rearrange("b p n -> p b n"))
            kmask_sb = ones.tile([H, nblk, BK], FP)
            nc.sync.dma_start(kmask_sb, kmask.rearrange("b p n -> p b n"))
            qkT_sb = ones.tile([128, 3, nblk, BQ + BK], BF)
            for a in range(3):
                nc.sync.dma_start(
                    qkT_sb[:, a], qkT[:, 128 * a:128 * (a + 1), :].rearrange(
                        "b p n -> p b n"))
            idb = ones.tile([128, 128], BF)
            make_identity(nc, idb)
            idf = ones.tile([128, 128], FP)
            make_identity(nc, idf)
            fT = ones.tile([128, 8, NQ], BF)
            nc.vector.memset(fT, 0.0)
            eps_ln = ones.tile([128, 1], FP)
            nc.vector.memset(eps_ln, 1e-5)
            eps_pt = ones.tile([128, 1], FP)
            nc.vector.memset(eps_pt, EPS)
            # absorb DMA waits for tensors later read as TensorScalarPtr
            # scalars (that ISA struct has a single sync-wait slot)
            absorb = ones.tile([1, 4], FP)
            nc.vector.tensor_copy(absorb[:, 0:1], rotq_sb[0:1, 0, 0:1])
            nc.vector.tensor_copy(absorb[:, 1:2], rotk_sb[0:1, 0, 0:1])
            nc.vector.tensor_copy(absorb[:, 2:3], bdz_sb[0:1, :])
            nc.vector.tensor_copy(absorb[:, 3:4], hwm_sb[0:1, 0:1])

            zp = ctx.enter_context(tc.tile_pool(name="zp", bufs=2))
            sb = ctx.enter_context(tc.tile_pool(name="sb", bufs=3))
            sc = ctx.enter_context(tc.tile_pool(name="sc", bufs=4))
            trb = ctx.enter_context(tc.tile_pool(name="trb", bufs=2,
                                                 space="PSUM"))
            med = ctx.enter_context(tc.tile_pool(name="med", bufs=2,
                                                 space="PSUM"))
            big = ctx.enter_context(tc.tile_pool(name="big", bufs=3,
                                                 space="PSUM"))
            lgt = ctx.enter_context(tc.tile_pool(name="lgt", bufs=1,
                                                 space="PSUM"))

            for blk in range(nblk):
                # ---- z loads ----
                z_rm = zp.tile([BK, BQ, CZ], BF, tag="zrm")      # [k, q, c]
                nc.sync.dma_start(z_rm, zb[blk].rearrange("q k c -> k q c"))
                z_tr = zp.tile([CZ, BQ * BK], BF, tag="ztr")     # [c, (q k)]
                for q in range(BQ):
                    ps_zt = trb.tile([128, 128], BF, tag="trb")
                    nc.tensor.transpose(ps_zt, z_rm[:, q, :], idb)
                    nc.scalar.activation(z_tr[:, BK * q:BK * (q + 1)], ps_zt,
                                         AF.Copy)

                # ---- z row stats -> rr [128k, 32q] ----
                zsq = sc.tile([BK, BQ, CZ], BF, tag="zsq")
                nc.vector.tensor_tensor(zsq, z_rm, z_rm, MUL)
                ssq = sc.tile([BK, BQ], FP, tag="ssq")
                nc.vector.tensor_reduce(ssq, zsq, AX, ADD)
                sm = sc.tile([BK, BQ], FP, tag="sm")
                nc.vector.tensor_reduce(sm, z_rm, AX, ADD)
                mm = sc.tile([BK, BQ], FP, tag="mm")
                nc.vector.tensor_scalar(mm, sm, 1.0 / CZ, None, MUL)
                var = sc.tile([BK, BQ], FP, tag="var")
                nc.vector.tensor_tensor(var, mm, mm, MUL)
                nc.vector.scalar_tensor_tensor(var, ssq, 1.0 / CZ, var,
                                               MUL, SUB)
                srt = sc.tile([BK, BQ], FP, tag="srt")
                nc.scalar.activation(srt, var, AF.Sqrt, bias=eps_ln[:BK])
                rr = sc.tile([BK, BQ], FP, tag="rr")
                nc.vector.reciprocal(rr, srt)

                # ---- projections ----
                ps_q = med.tile([BQ, NQW], FP, tag="med")
                ps_kv = big.tile([BK, NKV], FP, tag="big")
                ps_kp = big.tile([BK, NKP], FP, tag="big")
                for a in range(3):
                    qs = qkT_sb[:, a, blk, :BQ]
                    ks = qkT_sb[:, a, blk, BQ:]
                    nc.tensor.matmul(ps_q, qs, wcat_sb[:, a, :NQW],
                                     start=(a == 0), stop=(a == 2))
                    nc.tensor.matmul(ps_kv, ks, wcat_sb[:, a, NQW:NQW + NKV],
                                     start=(a == 0), stop=(a == 2))
                    nc.tensor.matmul(ps_kp, ks, wcat_sb[:, a, NQW + NKV:],
                                     start=(a == 0), stop=(a == 2))

                # ---- q rotation -> qcomb [32, h, 28] ----
                qcomb = sb.tile([BQ, H, 28], BF, tag="qcomb")
                nc.scalar.activation(
                    qcomb[:, :, :16],
                    ps_q[:, :192].rearrange("q (h c) -> q h c", h=H), AF.Copy)
                qR = rotq_sb[:, blk, :]
                qraw = ps_q[:, 192:].rearrange("q (hp j) -> q hp j", j=3)
                qdst = qcomb[:, :, 16:].rearrange("q h (p j) -> q h p j", p=PQK, j=3)
                tmpq = sc.tile([BQ, H * PQK], FP, tag="tmpq")
                for i in range(3):
                    nc.vector.tensor_scalar(tmpq, qraw[:, :, 0],
                                            qR[:, 3 * i:3 * i + 1], None, MUL)
                    nc.vector.scalar_tensor_tensor(
                        tmpq, qraw[:, :, 1], qR[:, 3 * i + 1:3 * i + 2],
                        tmpq, MUL, ADD)
                    nc.vector.scalar_tensor_tensor(
                        tmpq, qraw[:, :, 2],
                        qR[:, 3 * i + 2:3 * i + 3], tmpq, MUL, ADD)
                    nc.vector.tensor_scalar(qdst[:, :, :, i], tmpq,
                                            qR[:, 9 + i:10 + i], None, ADD)

                # ---- k rotation -> kcomb [128, h, 28], vpts, v_sb ----
                kcomb = sb.tile([BK, H, 28], BF, tag="kcomb")
                nc.scalar.activation(
                    kcomb[:, :, :16],
                    ps_kv[:, :192].rearrange("k (h c) -> k h c", h=H), AF.Copy)
                v_sb = sb.tile([BK, H, CH], BF, tag="v_sb")
                nc.scalar.activation(
                    v_sb, ps_kv[:, 192:].rearrange("k (h c) -> k h c", h=H),
                    AF.Copy)
                vpts = sb.tile([BK, H, PV, 3], BF, tag="vpts")
                kR = rotk_sb[:, blk, :]
                kraw = ps_kp.rearrange("k (hp j) -> k hp j", j=3)
                tmpk = sc.tile([BK, H * (PQK + PV)], FP, tag="tmpk")
                kdst = kcomb[:, :, 16:].rearrange("k h (p j) -> k h p j", p=PQK, j=3)
                for i in range(3):
                    nc.vector.tensor_scalar(tmpk, kraw[:, :, 0],
                                            kR[:, 3 * i:3 * i + 1], None, MUL)
                    nc.vector.scalar_tensor_tensor(
                        tmpk, kraw[:, :, 1], kR[:, 3 * i + 1:3 * i + 2],
                        tmpk, MUL, ADD)
                    nc.vector.scalar_tensor_tensor(
                        tmpk, kraw[:, :, 2],
                        kR[:, 3 * i + 2:3 * i + 3], tmpk, MUL, ADD)
                    tk = tmpk.rearrange("k (h p) -> k h p", p=PQK + PV)
                    nc.vector.tensor_scalar(kdst[:, :, :, i], tk[:, :, :PQK],
                                            kR[:, 9 + i:10 + i], None, ADD)
                    nc.vector.tensor_scalar(vpts[:, :, :, i], tk[:, :, PQK:],
                                            kR[:, 9 + i:10 + i], None, ADD)

                # ---- kadd = -0.5*hw*kn + mask  -> kaddT [12h, 128k] ----
                kpsq = sc.tile([BK, H, PQK * 3], FP, tag="kpsq")
                nc.vector.tensor_tensor(kpsq, kcomb[:, :, 16:],
                                        kcomb[:, :, 16:], MUL)
                kn = sc.tile([BK, H], FP, tag="kn")
                nc.vector.tensor_reduce(kn, kpsq, AX, ADD)
                kadd = sc.tile([BK, H], FP, tag="kadd")
                nc.vector.tensor_tensor(kadd, kn, hwm_sb[:BK, :], MUL)
                ps_kt = med.tile([128, 128], FP, tag="med")
                nc.tensor.transpose(ps_kt[:H, :], kadd, idf)
                kaddT = sb.tile([H, BK], FP, tag="kaddT")
                nc.vector.tensor_tensor(kaddT, ps_kt[:H, :],
                                        kmask_sb[:, blk, :], ADD)

                # ---- q/k assembly + fused logit matmuls ----
                lps = lgt.tile([128, 3, BK], FP, tag="lgt")
                for g in range(3):
                    ps_qa = trb.tile([128, 128], BF, tag="trb")
                    for h4 in range(4):
                        nc.tensor.transpose(
                            ps_qa[32 * h4:32 * h4 + 28,
                                  32 * h4:32 * h4 + 32],
                            qcomb[:, 4 * g + h4, :], idb[:BQ, :BQ],
                            tile_position=(0, 32 * h4))
                    qasm = sb.tile([128, 128], BF, tag="qasm")
                    nc.vector.memset(qasm, 0.0)
                    for h4 in range(4):
                        nc.scalar.activation(
                            qasm[32 * h4:32 * h4 + 28, 32 * h4:32 * h4 + 32],
                            ps_qa[32 * h4:32 * h4 + 28, 32 * h4:32 * h4 + 32],
                            AF.Copy,
                            scale=hwq_sb[32 * h4:32 * h4 + 28, g:g + 1])
                    ps_ka = trb.tile([128, 128], BF, tag="trb")
                    for h4 in range(4):
                        nc.tensor.transpose(ps_ka[32 * h4:32 * h4 + 28, :],
                                            kcomb[:, 4 * g + h4, :], idb,
                                            tile_position=(0, 32 * h4))
                    kasm = sb.tile([128, 128], BF, tag="kasm")
                    nc.vector.memset(kasm, 0.0)
                    for h4 in range(4):
                        nc.scalar.activation(
                            kasm[32 * h4:32 * h4 + 28, :],
                            ps_ka[32 * h4:32 * h4 + 28, :], AF.Copy)
                    nc.tensor.matmul(lps[:, g, :], qasm, kasm,
                                     start=True, stop=True)

                # ---- bbias raw [12h, (q k)] ----
                braw = sb.tile([H, BQ * BK], BF, tag="braw")
                for chk in range(8):
                    ps_bb = big.tile([H, 512], FP, tag="big")
                    nc.tensor.matmul(ps_bb, wz_sb[:, :H],
                                     z_tr[:, 512 * chk:512 * (chk + 1)],
                                     start=True, stop=True)
                    nc.scalar.activation(braw[:, 512 * chk:512 * (chk + 1)],
                                         ps_bb, AF.Copy)

                # rr^T replicated over h4 -> rrT [(h4 q), k]
                ps_rt = med.tile([128, 128], FP, tag="med")
                nc.tensor.transpose(ps_rt[:BQ, :], rr, idf)
                rrT = sb.tile([128, BK], FP, tag="rrT")
                for h4 in range(4):
                    nc.vector.tensor_copy(rrT[32 * h4:32 * (h4 + 1), :],
                                          ps_rt[:BQ, :])

                # ---- bias adds into logits ----
                for g in range(3):
                    bbr = sc.tile([128, BK], BF, tag="bbr")
                    nc.sync.dma_start(
                        bbr, braw[4 * g:4 * g + 4, :].rearrange(
                            "h (q k) -> h q k", q=BQ))
                    nc.vector.tensor_tensor(bbr, bbr, rrT, MUL)
                    nc.vector.tensor_tensor(lps[:, g, :], lps[:, g, :],
                                            bbr, ADD)
                    krep = sc.tile([128, BK], FP, tag="krep")
                    nc.sync.dma_start(krep, _bc(kaddT[4 * g:4 * g + 4, :],
                                                1, BQ))
                    nc.vector.tensor_tensor(lps[:, g, :], lps[:, g, :],
                                            krep, ADD)

                # ---- softmax over k ----
                a_sb = sb.tile([128, 3, BK], BF, tag="a_sb")
                den = sc.tile([128, 3], FP, tag="den")
                for g in range(3):
                    nmx = sc.tile([128, 1], FP, tag="nmx")
                    nc.vector.tensor_reduce(nmx, lps[:, g, :], AX,
                                            AluOpType.max, negate=True)
                    nc.scalar.activation(a_sb[:, g, :], lps[:, g, :], AF.Exp,
                                         bias=nmx, accum_out=den[:, g:g + 1])
                rden = sc.tile([128, 3], FP, tag="rden")
                nc.vector.reciprocal(rden, den)
                for g in range(3):
                    nc.vector.tensor_scalar(a_sb[:, g, :], a_sb[:, g, :],
                                            rden[:, g:g + 1], None, MUL)

                # ---- aT [128k, h, 32q] and rr-folded arT ----
                aT = sb.tile([BK, H, BQ], BF, tag="aT")
                for g in range(3):
                    for h4 in range(4):
                        ps_at = trb.tile([128, 128], BF, tag="trb")
                        nc.tensor.transpose(
                            ps_at[:, :BQ],
                            a_sb[32 * h4:32 * (h4 + 1), g, :],
                            idb[32 * h4:32 * (h4 + 1), 32 * h4:32 * (h4 + 1)],
                            tile_position=(32 * h4, 0))
                        nc.scalar.activation(aT[:, 4 * g + h4, :],
                                             ps_at[:, :BQ], AF.Copy)
                arT = sb.tile([BK, H, BQ], BF, tag="arT")
                nc.vector.tensor_tensor(arT, aT, _bc(rr, 1, H), MUL)

                # ---- o / o_pt ----
                ps_o = med.tile([BQ, H, 16 + PV * 3], FP, tag="med")
                for h in range(H):
                    nc.tensor.matmul(ps_o[:, h, :16], aT[:, h, :],
                                     v_sb[:, h, :], start=True, stop=True)
                    nc.tensor.matmul(
                        ps_o[:, h, 16:], aT[:, h, :],
                        vpts[:, h, :, :].rearrange("k v j -> k (v j)"),
                        start=True, stop=True)

                # ---- feats row-major [32, 576] ----
                frm = sb.tile([BQ, NF0], BF, tag="frm")
                nc.scalar.activation(
                    frm[:, :192].rearrange("q (h c) -> q h c", h=H),
                    ps_o[:, :, :16], AF.Copy)
                gsub = sc.tile([BQ, H * PV, 3], FP, tag="gsub")
                gsrc = ps_o[:, :, 16:].rearrange("q h (v j) -> q h v j", v=PV, j=3)
                for i in range(3):
                    nc.vector.tensor_scalar(gsub[:, :, i], gsrc[:, :, :, i],
                                            qR[:, 9 + i:10 + i], None, SUB)
                optl = frm[:, 192:480].rearrange("q (hv j) -> q hv j", j=3)
                tmpo = sc.tile([BQ, H * PV], FP, tag="tmpo")
                for j in range(3):
                    nc.vector.tensor_scalar(tmpo, gsub[:, :, 0],
                                            qR[:, j:j + 1], None, MUL)
                    nc.vector.scalar_tensor_tensor(
                        tmpo, gsub[:, :, 1], qR[:, 3 + j:4 + j], tmpo,
                        MUL, ADD)
                    nc.vector.scalar_tensor_tensor(
                        optl[:, :, j], gsub[:, :, 2], qR[:, 6 + j:7 + j],
                        tmpo, MUL, ADD)
                osq = sc.tile([BQ, H * PV, 3], FP, tag="osq")
                nc.vector.tensor_tensor(osq, optl, optl, MUL)
                osum = sc.tile([BQ, H * PV], FP, tag="osum")
                nc.vector.tensor_reduce(osum, osq, AX, ADD)
                nc.scalar.activation(
                    frm[:, 480:], osum, AF.Sqrt, bias=eps_pt[:BQ])

                # ---- o_pair: w1 then Wdz~ ----
                ps_w1 = big.tile([CZ, BQ, H], FP, tag="big")
                for q in range(BQ):
                    nc.tensor.matmul(ps_w1[:, q, :], z_rm[:, q, :],
                                     arT[:, :, q], start=True, stop=True)
                w1 = sb.tile([CZ, BQ, H], BF, tag="w1")
                nc.scalar.activation(w1, ps_w1, AF.Copy)
                ps_p = med.tile([BQ, BQ, H], FP, tag="med")
                nc.tensor.matmul(ps_p.rearrange("c q h -> c (q h)"),
                                 wz_sb[:, 12:],
                                 w1.rearrange("c q h -> c (q h)"),
                                 start=True, stop=True)
                for h in range(H):
                    r0 = NF0 + 32 * h
                    nc.vector.tensor_scalar(
                        fT[(r0 % 128):(r0 % 128) + 32, r0 // 128,
                           BQ * blk:BQ * (blk + 1)],
                        ps_p[:, :, h], bdz_sb, None, ADD)

                # ---- feats transposes into fT rows 0..576 ----
                for t5 in range(5):
                    w = 128 if t5 < 4 else 64
                    ps_ft = trb.tile([128, 128], BF, tag="trb")
                    nc.tensor.transpose(ps_ft[:w, :BQ],
                                        frm[:, 128 * t5:128 * t5 + w],
                                        idb[:BQ, :BQ])
                    nc.scalar.activation(fT[:w, t5, BQ * blk:BQ * (blk + 1)],
                                         ps_ft[:w, :BQ], AF.Copy)

            # ---- final out projection: outT = Wout^T @ feats^T ----
            for m in range(3):
                ps_out = big.tile([128, NQ], FP, tag="big")
                for kc in range(8):
                    nc.tensor.matmul(ps_out,
                                     wout_sb[:, kc, 128 * m:128 * (m + 1)],
                                     fT[:, kc, :],
                                     start=(kc == 0), stop=(kc == 7))
                osb = sb.tile([128, NQ], FP, tag="osb")
                nc.scalar.activation(osb, ps_out, AF.Copy)
                nc.sync.dma_start(outT[m], osb)
    nc.compile()
    return nc


# ----------------------------------------------------------------------
# host side
# ----------------------------------------------------------------------

def _softplus(x):
    return np.logaddexp(f32(0.0), x.astype(f32)).astype(f32)


def prep_inputs(s, z, trans, rots, s_mask, key_idx,
                ln_s_g, ln_s_b, ln_z_g, ln_z_b,
                Wq, Wk, Wv, Wqp, Wkvp, Wb, Wdz, head_weights, Wout,
                nblk=BLK, ncores=NCORES):
    """Returns list of per-core input dicts."""
    s = np.asarray(s, f32); z = np.asarray(z, f32)
    trans = np.asarray(trans, f32); rots = np.asarray(rots, f32)
    s_mask = np.asarray(s_mask, f32)
    key_idx = np.asarray(key_idx).astype(np.int64)
    args = [np.asarray(a, f32) for a in
            (ln_s_g, ln_s_b, ln_z_g, ln_z_b, Wq, Wk, Wv, Wqp, Wkvp, Wb, Wdz,
             head_weights, Wout)]
    (g_s, b_s, g_z, b_z, Wq, Wk, Wv, Wqp, Wkvp, Wb, Wdz, hw_raw, Wout) = args

    s1 = f32(np.sqrt(1.0 / (3 * CH)))
    s2 = f32(np.sqrt(1.0 / 3.0))
    hw = _softplus(hw_raw) * f32(np.sqrt(1.0 / (3 * (PQK * 9.0 / 2))))

    # LN of s on host
    mu = s.mean(-1, keepdims=True)
    va = ((s - mu) ** 2).mean(-1, keepdims=True)
    sN = ((s - mu) / np.sqrt(va + f32(1e-5)) * g_s + b_s).astype(f32)

    nb_tot = B * NB
    # qkT: [blk, CS, BQ+BK]
    q_side = sN.reshape(B, NB, BQ, CS).transpose(0, 1, 3, 2)     # [B,NB,CS,BQ]
    k_side = sN[:, key_idx].transpose(0, 1, 3, 2)                # [B,NB,CS,BK]
    qkT_all = np.concatenate([q_side, k_side], -1).reshape(
        nb_tot, CS, BQ + BK).astype(bf16)

    # weights
    wcat = np.zeros((CS, NQW + NKV + NKP), f32)
    wcat[:, :192] = Wq * s1
    wcat[:, 192:336] = Wqp
    wcat[:, 336:528] = Wk
    wcat[:, 528:720] = Wv
    wcat[:, 720:] = Wkvp
    wcat = wcat.reshape(3, 128, -1).astype(bf16)

    P = np.eye(CZ, dtype=f32) - f32(1.0 / CZ)
    wz = np.zeros((CZ, 44), f32)
    wz[:, :H] = P @ (g_z[:, None] * Wb) * s2
    wz[:, H:12] = 0.0
    wz[:, 12:] = P @ (g_z[:, None] * Wdz)
    wz = wz.astype(bf16)

    wout_p = np.zeros((1024, CS), f32)
    wout_p[:FEAT_TOT] = Wout
    wout_p = wout_p.reshape(8, 128, CS).astype(bf16)

    # rotations: cols 3i+j = R[i,j], 9+i = t[i]
    rq = np.concatenate([rots.reshape(B * N, 9), trans.reshape(B * N, 3)],
                        -1).reshape(B, N, 12)
    rotq_all = rq.reshape(B, NB, BQ, 12).reshape(nb_tot, BQ, 12)
    rotk_all = rq[:, key_idx].reshape(nb_tot, BK, 12)

    hwq_t = np.zeros((128, 4), f32)
    for g in range(3):
        for h4 in range(4):
            hwq_t[32 * h4:32 * h4 + 16, g] = 1.0
            hwq_t[32 * h4 + 16:32 * h4 + 28, g] = hw[4 * g + h4]
    for h4 in range(4):
        hwq_t[32 * h4:32 * h4 + 28, 3] = 1.0

    hwm_t = np.broadcast_to(-0.5 * hw, (128, H)).astype(f32).copy()

    km = s_mask[:, key_idx]                                      # [B,NB,BK]
    kmask_all = (-INF * (1.0 - km))[:, :, None, :]
    kmask_all = np.broadcast_to(kmask_all, (B, NB, H, BK)).reshape(
        nb_tot, H, BK).astype(f32).copy()

    bdz_t = (b_z @ Wdz).astype(f32).reshape(BQ, 1)

    zb_all = z.reshape(nb_tot, BQ, BK, CZ).astype(bf16)

    in_maps = []
    for c in range(ncores):
        sl = slice(c * nblk, (c + 1) * nblk)
        in_maps.append(dict(
            zb=np.ascontiguousarray(zb_all[sl]),
            qkT=np.ascontiguousarray(qkT_all[sl]),
            wcat=wcat, wz=wz, wout=wout_p,
            rotq=np.ascontiguousarray(rotq_all[sl]),
            rotk=np.ascontiguousarray(rotk_all[sl]),
            hwq=hwq_t, hwm=hwm_t,
            kmask=np.ascontiguousarray(kmask_all[sl]),
            bdz=bdz_t,
        ))
    return in_maps


FEAT_TOT = 960


def assemble_out(results, nblk=BLK):
    """results: list of per-core {'outT': [3,128,nblk*32]} -> [B,N,CS]."""
    out = np.zeros((B * NB, BQ, CS), f32)
    for c, r in enumerate(results):
        oT = r["outT"]                                  # [3,128,nblk*32]
        o = oT.reshape(CS, nblk, BQ).transpose(1, 2, 0)  # [nblk, BQ, CS]
        out[c * nblk:(c + 1) * nblk] = o
    return out.reshape(B, N, CS)


def kernel(s, z, trans, rots, s_mask, key_idx,
           ln_s_g, ln_s_b, ln_z_g, ln_z_b,
           Wq, Wk, Wv, Wqp, Wkvp, Wb, Wdz, head_weights, Wout,
           _trace=False):
    in_maps = prep_inputs(s, z, trans, rots, s_mask, key_idx,
                          ln_s_g, ln_s_b, ln_z_g, ln_z_b,
                          Wq, Wk, Wv, Wqp, Wkvp, Wb, Wdz, head_weights, Wout)
    nc = build_nc()
    import time
    t0 = time.perf_counter()
    res = run_bass_kernel_spmd(nc, in_maps, core_ids=list(range(NCORES)),
                               trace=False)
    t1 = time.perf_counter()
    kernel._last_exec_ns = (res.exec_time_ns if res.exec_time_ns is not None
                            else int((t1 - t0) * 1e9))
    return assemble_out([r for r in res.results])
